# revision 43
# baseline (speedup 1.0000x reference)
"""CrossAttnBlock TRN2 kernel: 8-way (batch x l-half) sharded, collective-free.

Reference math (b=4, c=64, h=64, w=32, dim=256, HEADS=8, l=h*w=2048):
  zf = z.reshape(b, dim, l).T            # [b, l, dim]
  q  = x.reshape(b, c, l).T              # [b, l, c]
  k  = (zf @ Wk + bk) -> [b, H, l, c];  v likewise
  S  = q @ k.T / sqrt(c); A = softmax(S, -1); P = A @ v
  out = (P heads-concat) @ Wo + bo       # [b, l, c]
  return x + out.reshape(b, c, h, w)     # raw-memory reinterpretation

Per-core (core = bi*2 + half): full K/V projection for batch bi, attention +
out-proj for l rows [half*1024, (half+1)*1024).

Phase C processes head PAIRS with row-tiled score matmuls (heads 2p/2p+1 in
partition halves of kT run concurrently in the PE array), software-pipelined
so each iteration's AV matmuls trail one iteration behind the scores (the exp
latency hides under the next score pair). The softmax exp splits between the
Scalar engine (table exp) and the Vector engine (Schraudolph bit-trick exp:
bf16 bits = int16(s*A + B), validated exact on HW). Softmax denominators come
from a ones-augmented V column. Phase D (per-head normalize + out-proj +
residual) runs as a tail: denominator reciprocals are computed row-wise once,
transposed to [l, h] via the DVE 32x32 block transpose + one partition-shuffle
DMA per 32-row group. bo and bv fold into the host-side residual (out-proj is
linear); bk is added on the kT drain path.
"""
import ml_dtypes
import numpy as np

import concourse.bass as bass
import concourse.mybir as mybir
import concourse.tile as tile
from concourse import bacc
from concourse.bass_utils import run_bass_kernel_spmd

F32 = mybir.dt.float32
BF16 = mybir.dt.bfloat16
I16 = mybir.dt.int16
I8 = mybir.dt.int8
F8 = mybir.dt.float8e4

B, C, H, W = 4, 64, 64, 32
DIM = 256
HEADS = 8
L = H * W            # 2048
LH = L // 2          # 1024 per core
INNER = HEADS * C    # 512
N_CORES = 8
NMT = L // 128       # 16 m-tiles
NLS = LH // 128      # 8 l-subtiles

SCALE = float(C) ** -0.5
# Schraudolph exp in bf16 bits: bf16(exp(s*SCALE)) ~ int16(s*EXP_A + EXP_B)
EXP_A = (2.0 ** 7 / float(np.log(2.0))) * SCALE
EXP_B = 16256.0 - 7.4
# same trick in fp8e4m3 bits (bias 7, 3 mantissa bits)
EXP_A8 = (2.0 ** 3 / float(np.log(2.0))) * SCALE
EXP_B8 = 56.0 - 0.46

_CACHE = {}


def build_nc():
    nc = bacc.Bacc("TRN2", target_bir_lowering=False, debug=False,
                   num_devices=N_CORES)
    zb = nc.dram_tensor("zb", [DIM, L], F8, kind="ExternalInput")
    Wk = nc.dram_tensor("Wk", [DIM, INNER], F8, kind="ExternalInput")
    Wv = nc.dram_tensor("Wv", [DIM, INNER], F8, kind="ExternalInput")
    xq = nc.dram_tensor("xq", [C, LH], BF16, kind="ExternalInput")
    xr = nc.dram_tensor("xr", [128, NLS, C], F32, kind="ExternalInput")
    Wo = nc.dram_tensor("Wo", [C, HEADS, C], BF16, kind="ExternalInput")
    bk = nc.dram_tensor("bk", [128, 4], F32, kind="ExternalInput")
    ones8 = nc.dram_tensor("ones8", [128, 128], F8, kind="ExternalInput")
    OUT = nc.dram_tensor("out", [LH, C], F32, kind="ExternalOutput")

    AF = mybir.ActivationFunctionType
    OP = mybir.AluOpType

    with tile.TileContext(nc) as tc:
        with (
            tc.tile_pool(name="const", bufs=1) as cp,
            tc.tile_pool(name="es", bufs=6) as ep,
            tc.tile_pool(name="sm", bufs=3) as sp,
            tc.tile_pool(name="ps_s", bufs=4, space="PSUM") as ps_s,
            tc.tile_pool(name="ps_pt", bufs=4, space="PSUM") as ps_pt,
        ):
            # ---- inputs to SBUF (z + Wk first so phase A starts early) ----
            z_sb = cp.tile([128, 2, L], F8, tag="z")
            zr = zb.rearrange("(d p) l -> p d l", d=2)
            nc.sync.dma_start(out=z_sb[:, :, 0:512], in_=zr[:, :, 0:512])
            wk_sb = cp.tile([128, 2, INNER], F8, tag="wk")
            nc.sync.dma_start(out=wk_sb,
                              in_=Wk.rearrange("(d p) i -> p d i", d=2))
            nc.sync.dma_start(out=z_sb[:, :, 512:L], in_=zr[:, :, 512:L])
            bk_sb = cp.tile([128, 4], F32, tag="bk")
            nc.sync.dma_start(out=bk_sb, in_=bk[:, :])
            wv_sb = cp.tile([128, 2, INNER], F8, tag="wv")
            nc.sync.dma_start(out=wv_sb,
                              in_=Wv.rearrange("(d p) i -> p d i", d=2))
            x_sb = cp.tile([128, LH], BF16, tag="x")
            nc.sync.dma_start(out=x_sb[0:C, :], in_=xq[:, :])
            nc.sync.dma_start(out=x_sb[C:2 * C, :], in_=xq[:, :])
            xr_sb = cp.tile([128, NLS, C], F32, tag="xr")
            nc.sync.dma_start(out=xr_sb, in_=xr[:, :, :])
            wo_sb = cp.tile([C, HEADS, C], BF16, tag="wo")
            nc.sync.dma_start(out=wo_sb, in_=Wo[:, :, :])

            kT = [cp.tile([128, L], BF16, tag=f"kT{t}", name=f"kT{t}")
                  for t in range(4)]
            v_sb = cp.tile([128, NMT, HEADS, C + 2], F8, tag="v")
            nc.sync.dma_start(
                out=v_sb[:, :, :, C:C + 1],
                in_=ones8.rearrange("p (a b c) -> p a b c", a=NMT, b=HEADS))
            pt_sb = [cp.tile([C + 1, LH], BF16, tag=f"pt{h}", name=f"pt{h}")
                     for h in range(HEADS)]
            # denominators, rows 0-7 used; padded to 32 partitions for the
            # DVE block transpose
            sums_sb = cp.tile([32, LH], BF16, tag="sums")
            nc.gpsimd.memset(sums_sb, 1.0)

            # ---- Phase A: kT[ci, m] = (Wk^T @ zf^T) + bk ----
            for s in range(4):
                for t in range(4):
                    pk = ps_s.tile([128, 512], F32, tag="s", name="pk")
                    nc.tensor.matmul(
                        pk,
                        wk_sb[:, :, t * 128:(t + 1) * 128],
                        z_sb[:, :, s * 512:(s + 1) * 512],
                        start=True, stop=True,
                        perf_mode=mybir.MatmulPerfMode.DoubleRow)
                    dst = kT[t][:, s * 512:(s + 1) * 512]
                    if (s * 4 + t) % 2 == 0:
                        nc.vector.tensor_scalar(
                            out=dst, in0=pk, scalar1=bk_sb[:, t:t + 1],
                            scalar2=None, op0=OP.add)
                    else:
                        nc.scalar.activation(
                            out=dst, in_=pk, func=AF.Identity,
                            bias=bk_sb[:, t:t + 1], scale=1.0)

            # ---- Phase B: v[m, h, ci] = zf @ Wv (ones col appended) ----
            for s in range(NMT):
                pv = ps_s.tile([128, 512], F32, tag="s", name="pv")
                nc.tensor.matmul(
                    pv,
                    z_sb[:, :, s * 128:(s + 1) * 128],
                    wv_sb[:, :, :],
                    start=True, stop=True,
                    perf_mode=mybir.MatmulPerfMode.DoubleRow)
                pvr = pv.rearrange("p (h c) -> p h c", h=HEADS)
                if s % 2 == 0:
                    nc.vector.tensor_copy(out=v_sb[:, s, :, 0:C], in_=pvr)
                else:
                    nc.scalar.activation(out=v_sb[:, s, :, 0:C], in_=pvr,
                                         func=AF.Copy)

            # ---- Phase C: attention, software-pipelined (AV trails scores
            #      by one iteration so exp latency hides) ----
            iters = [(lh, p, mt)
                     for lh in range(2) for p in range(4) for mt in range(NMT)]
            pend = []          # (ptp_e, ptp_o, es_e, es_ob, mt, lh, p)
            cur = {}

            NMP = NMT // 2
            DR = mybir.MatmulPerfMode.DoubleRow

            def emit_av(st):
                ptp_e, ptp_o, es_pe, es_po, mtp, lh, p = st
                nc.tensor.matmul(
                    ptp_e, v_sb[:, 2 * mtp:2 * mtp + 2, 2 * p, 0:C + 1],
                    es_pe, start=(mtp == 0),
                    stop=(mtp == NMP - 1), perf_mode=DR)
                nc.tensor.matmul(
                    ptp_o, v_sb[:, 2 * mtp:2 * mtp + 2, 2 * p + 1, 0:C + 1],
                    es_po, start=(mtp == 0),
                    stop=(mtp == NMP - 1), perf_mode=DR)

            # denominator transpose chain, per l-half: [8, 512] -> [l, 4, 8]
            # via DVE 32x32 block transpose + partition-shuffle DMAs + recip
            str_ = [cp.tile([32, 512], BF16, tag=f"str{i}", name=f"str{i}")
                    for i in range(2)]
            sums_t = cp.tile([128, NLS, 8], BF16, tag="sumst")
            recip_all = cp.tile([128, NLS, 8], F32, tag="recall")

            def emit_recip(lh):
                lo = lh * 512
                nc.vector.transpose(out=str_[lh], in_=sums_sb[:, lo:lo + 512])
                for j in range(4):
                    nc.sync.dma_start(
                        out=sums_t[32 * j:32 * (j + 1), 4 * lh:4 * lh + 4, :],
                        in_=str_[lh].rearrange("p (ls j h32) -> p ls j h32",
                                               ls=4, j=4)[:, :, j, 0:8])
                nc.vector.reciprocal(out=recip_all[:, 4 * lh:4 * lh + 4, :],
                                     in_=sums_t[:, 4 * lh:4 * lh + 4, :])

            def emit_drain(lh, p, tile_e, tile_o):
                lo = lh * 512
                he, ho = 2 * p, 2 * p + 1
                nc.vector.tensor_copy(out=pt_sb[he][:, lo:lo + 512],
                                      in_=tile_e)
                nc.vector.tensor_copy(out=pt_sb[ho][:, lo:lo + 512],
                                      in_=tile_o)
                nc.sync.dma_start(out=sums_sb[he:he + 1, lo:lo + 512],
                                  in_=pt_sb[he][C:C + 1, lo:lo + 512])
                nc.sync.dma_start(out=sums_sb[ho:ho + 1, lo:lo + 512],
                                  in_=pt_sb[ho][C:C + 1, lo:lo + 512])
                if p == 3:
                    emit_recip(lh)

            it = 0
            for lh, p, mt in iters:
                lo = lh * 512
                ms = mt * 128
                se = ps_s.tile([128, 512], F32, tag="s", name="se")
                so = ps_s.tile([128, 512], F32, tag="s", name="so")
                nc.tensor.matmul(se, kT[p][0:64, ms:ms + 128],
                                 x_sb[0:64, lo:lo + 512],
                                 start=True, stop=True)
                nc.tensor.matmul(so, kT[p][64:128, ms:ms + 128],
                                 x_sb[64:128, lo:lo + 512],
                                 start=True, stop=True)
                if mt % 2 == 0:
                    es_pe = ep.tile([128, 2, 512], F8, tag="es", name="espe")
                    es_po = ep.tile([128, 2, 512], F8, tag="es", name="espo")
                k = mt % 2
                nc.scalar.activation(out=es_pe[:, k, :], in_=se, func=AF.Exp,
                                     scale=SCALE)
                if it % 8 == 7:
                    nc.scalar.activation(out=es_po[:, k, :], in_=so,
                                         func=AF.Exp, scale=SCALE)
                else:
                    nc.vector.tensor_scalar(
                        out=es_po[:, k, :].bitcast(I8), in0=so,
                        scalar1=EXP_A8, scalar2=EXP_B8,
                        op0=OP.mult, op1=OP.add)
                if mt == 0:
                    cur = {"e": ps_pt.tile([C + 1, 512], F32, tag="pt",
                                           name="ptpe"),
                           "o": ps_pt.tile([C + 1, 512], F32, tag="pt",
                                           name="ptpo")}
                if mt % 2 == 1:
                    pend.append((cur["e"], cur["o"], es_pe, es_po,
                                 mt // 2, lh, p))
                    if len(pend) > 1:
                        st = pend.pop(0)
                        emit_av(st)
                        if st[4] == NMP - 1:
                            emit_drain(st[5], st[6], st[0], st[1])
                it += 1
            for st in pend:
                emit_av(st)
                if st[4] == NMP - 1:
                    emit_drain(st[5], st[6], st[0], st[1])

            # ---- Phase D: normalize + out-proj + residual (tail; recip
            #      chains already emitted per l-half inside phase C) ----
            fin_all = cp.tile([128, NLS, C], F32, tag="finall")
            for ls in range(NLS):
                lo = ls * 128
                po = ps_s.tile([128, HEADS, C], F32, tag="s", name="po")
                for h in range(HEADS):
                    nc.tensor.matmul(
                        po[:, h, :],
                        pt_sb[h][0:C, lo:lo + 128],
                        wo_sb[:, h, :],
                        start=True, stop=True)
                tmp = sp.tile([128, HEADS, C], F32, tag="tmp", name="tmp")
                for h2 in range(2):
                    nc.scalar.activation(
                        out=tmp[:, h2, :], in_=po[:, h2, :], func=AF.Identity,
                        scale=recip_all[:, ls, h2:h2 + 1], bias=0.0)
                rb = recip_all[:, ls, 2:8].rearrange("p (h o) -> p h o", o=1)
                nc.vector.tensor_tensor(
                    out=tmp[:, 2:8, :], in0=po[:, 2:8, :],
                    in1=rb.broadcast_to([128, 6, C]), op=OP.mult)
                t1 = sp.tile([128, 4, C], F32, tag="t1", name="t1")
                nc.vector.tensor_tensor(out=t1, in0=tmp[:, 0:4, :],
                                        in1=tmp[:, 4:8, :], op=OP.add)
                t2 = sp.tile([128, 2, C], F32, tag="t2", name="t2")
                nc.gpsimd.tensor_tensor(out=t2, in0=t1[:, 0:2, :],
                                        in1=t1[:, 2:4, :], op=OP.add)
                fin = fin_all[:, ls, :]
                nc.gpsimd.tensor_tensor(out=fin, in0=t2[:, 0, :],
                                        in1=t2[:, 1, :], op=OP.add)
                nc.gpsimd.tensor_tensor(out=fin, in0=fin,
                                        in1=xr_sb[:, ls, :], op=OP.add)
            nc.sync.dma_start(
                out=OUT.rearrange("(ls p) c -> p ls c", ls=NLS), in_=fin_all)

    nc.compile()
    return nc


def kernel(x, z, Wk, bk, Wv, bv, Wo, bo):
    x = np.ascontiguousarray(x, dtype=np.float32)
    z = np.ascontiguousarray(z, dtype=np.float32)
    Wk = np.asarray(Wk, np.float32)
    Wv = np.asarray(Wv, np.float32)
    Wo = np.asarray(Wo, np.float32)
    bk = np.asarray(bk, np.float32)
    bv = np.asarray(bv, np.float32)
    bo = np.asarray(bo, np.float32)
    if "nc" not in _CACHE:
        _CACHE["nc"] = build_nc()
    nc = _CACHE["nc"]
    # out-proj is linear, so the constant V bias folds into the residual:
    # ((P + bv*d)/d) @ Wo + bo = (P/d) @ Wo + (bv @ Wo + bo)
    res_bias = bv @ Wo + bo                      # [C]
    shared = {
        "Wk": np.ascontiguousarray(Wk.astype(mybir.dt.np(F8))),
        "Wv": np.ascontiguousarray(Wv.astype(mybir.dt.np(F8))),
        "Wo": np.ascontiguousarray(
            Wo.reshape(HEADS, C, C).transpose(1, 0, 2)
            .astype(ml_dtypes.bfloat16)),
        "bk": np.ascontiguousarray(bk.reshape(4, 128).T),
        "ones8": np.ones((128, 128), mybir.dt.np(F8)),
    }
    in_maps = []
    for core in range(N_CORES):
        bi, half = core // 2, core % 2
        xi = x[bi].reshape(C, L)
        in_maps.append({
            "xq": np.ascontiguousarray(
                xi[:, half * LH:(half + 1) * LH].astype(ml_dtypes.bfloat16)),
            "xr": np.ascontiguousarray(
                x[bi].reshape(-1)[half * LH * C:(half + 1) * LH * C]
                .reshape(NLS, 128, C).transpose(1, 0, 2) + res_bias),
            "zb": np.ascontiguousarray(
                z[bi].reshape(DIM, L).astype(mybir.dt.np(F8))),
            **shared,
        })
    _CACHE["in_maps"] = in_maps
    if "warm" not in _CACHE:
        # First execution after NEFF load runs with cold DMA rings and
        # wildly different timing; settle the device before the real run.
        run_bass_kernel_spmd(nc, in_maps, list(range(N_CORES)))
        _CACHE["warm"] = True
    res = run_bass_kernel_spmd(nc, in_maps, list(range(N_CORES)))
    full = np.empty((B, L * C), dtype=np.float32)
    for core in range(N_CORES):
        bi, half = core // 2, core % 2
        full[bi, half * LH * C:(half + 1) * LH * C] = \
            res.results[core]["out"].reshape(-1)
    return full.reshape(B, C, H, W)
